# revision 2
# baseline (speedup 1.0000x reference)
"""EntropyBottleneck forward kernel for Trainium2 (8 NeuronCores, data-parallel).

Math: with the per-channel gate params f == 0 (always true for this problem's
inputs), each _logits_cumulative layer is affine, so the whole 4-layer chain
collapses to t = a_c * x + d_c per channel c. The likelihood then is

    lik = | sigmoid(s*(t+h)) - sigmoid(s*(t-h)) |,  s = -sign(2t), h = a_c/2 > 0
        =   sigmoid(-|t| + h) - sigmoid(-|t| - h)   (>= 0, then clipped at 1e-9)

Device work per element: o = x + n (gpsimd), |a*o + d| (ACT Abs w/ per-partition
scale+bias), two sigmoids (ACT w/ per-partition bias), subtract (gpsimd),
clip (DVE, fused with PSUM->SBUF evac). Channels are moved onto the partition
axis with TensorE 128x128 transposes (and back), so all per-channel params are
plain per-partition [128,1] scale/bias vectors and DMAs stay fully contiguous.

Sharding: data-parallel over points N across the 8 cores; tiny params
replicated; no cross-core communication.
"""

import numpy as np

N_TOTAL = 500000
C = 64
N_CORES = 8
ROWS_PER_CORE = N_TOTAL // N_CORES          # 62500
ELEMS = ROWS_PER_CORE * C                   # 4,000,000 per core
CHUNKS = ELEMS // 128                       # 31,250 rows of the [CHUNKS,128] view
G_FULL = 16                                 # 128-col blocks per full tile
TILE_F = G_FULL * 128                       # 2048
CHUNKS_PER_TILE = TILE_F                    # a [128, 2048] tile covers 2048 chunks
N_FULL_TILES = CHUNKS // CHUNKS_PER_TILE    # 15
G_PART = 4
TILE_F2 = G_PART * 128                      # 512; covers 512 chunks
TAIL_CHUNKS = CHUNKS - N_FULL_TILES * CHUNKS_PER_TILE - TILE_F2  # 18

_CACHE: dict = {}


def _softplus64(x):
    return np.log1p(np.exp(-np.abs(x))) + np.maximum(x, 0.0)


def _collapse_affine(inputs):
    """Fold the 4 affine layers into per-channel (a, d) in float64."""
    alpha = None
    beta = None
    for i in range(4):
        W = _softplus64(np.asarray(inputs[f"m{i}"], dtype=np.float64))  # (C, fo, fi)
        bb = np.asarray(inputs[f"b{i}"], dtype=np.float64)[:, :, 0]     # (C, fo)
        if i == 0:
            alpha = W[:, :, 0]
            beta = bb
        else:
            alpha = np.einsum("cij,cj->ci", W, alpha)
            beta = np.einsum("cij,cj->ci", W, beta) + bb
    return alpha[:, 0], beta[:, 0]  # (C,), (C,)


def _build_bass():
    import concourse.bacc as bacc
    import concourse.mybir as mybir
    from concourse.mybir import ActivationFunctionType as AF
    from concourse.mybir import AluOpType as ALU
    from concourse.tile import TileContext

    f32 = mybir.dt.float32
    nc = bacc.Bacc("TRN2", target_bir_lowering=False, debug=False,
                   enable_asserts=False, num_devices=N_CORES)

    x_d = nc.dram_tensor("x", [CHUNKS, 128], f32, kind="ExternalInput")
    n_d = nc.dram_tensor("n", [CHUNKS, 128], f32, kind="ExternalInput")
    prm_d = nc.dram_tensor("prm", [128, 4], f32, kind="ExternalInput")
    idn_d = nc.dram_tensor("idn", [128, 128], f32, kind="ExternalInput")
    o_d = nc.dram_tensor("o", [CHUNKS, 128], f32, kind="ExternalOutput")
    lik_d = nc.dram_tensor("lik", [CHUNKS, 128], f32, kind="ExternalOutput")

    with TileContext(nc) as tc:
        with (
            tc.tile_pool(name="const", bufs=1) as constp,
            tc.tile_pool(name="io", bufs=3) as iop,
            tc.tile_pool(name="work", bufs=2) as workp,
            tc.tile_pool(name="pin", bufs=2, space="PSUM") as pinp,
            tc.tile_pool(name="pout", bufs=2, space="PSUM") as poutp,
        ):
            prm = constp.tile([128, 4], f32)
            nc.sync.dma_start(prm[:], prm_d[:, :])
            idn = constp.tile([128, 128], f32)
            nc.sync.dma_start(idn[:], idn_d[:, :])
            a_ap = prm[:, 0:1]
            d_ap = prm[:, 1:2]
            h_ap = prm[:, 2:3]
            nh_ap = prm[:, 3:4]

            def do_tile(c0, g):
                """Process chunks [c0, c0 + g*128) as a [128, g*128] tile."""
                F = g * 128
                nch = F  # chunks covered
                xs = x_d[c0:c0 + nch, :].rearrange("(q g) j -> q (g j)", q=128)
                ns = n_d[c0:c0 + nch, :].rearrange("(q g) j -> q (g j)", q=128)
                os = o_d[c0:c0 + nch, :].rearrange("(q g) j -> q (g j)", q=128)
                ls = lik_d[c0:c0 + nch, :].rearrange("(q g) j -> q (g j)", q=128)

                xt = iop.tile([128, F], f32, tag="xt")
                nc.sync.dma_start(xt[:], xs)
                nt = iop.tile([128, F], f32, tag="nt")
                nc.sync.dma_start(nt[:], ns)

                ot = iop.tile([128, F], f32, tag="ot")
                nc.gpsimd.tensor_tensor(ot[:], xt[:], nt[:], ALU.add)
                nc.sync.dma_start(os, ot[:])

                at = workp.tile([128, F], f32, tag="at")
                HB = g // 2  # 128-blocks per PSUM half-tile
                HF = HB * 128
                for h in range(2):
                    pin = pinp.tile([128, HF], f32, tag="pin")
                    for k in range(HB):
                        nc.tensor.transpose(
                            pin[:, k * 128:(k + 1) * 128],
                            ot[:, h * HF + k * 128: h * HF + (k + 1) * 128],
                            idn[:],
                        )
                    # at = |a * oT + d|, per-partition scale/bias
                    nc.scalar.activation(at[:, h * HF:(h + 1) * HF], pin[:],
                                         AF.Abs, bias=d_ap, scale=a_ap)

                pu = workp.tile([128, F], f32, tag="pu")
                nc.scalar.activation(pu[:], at[:], AF.Sigmoid, bias=h_ap, scale=-1.0)
                pl = workp.tile([128, F], f32, tag="pl")
                nc.scalar.activation(pl[:], at[:], AF.Sigmoid, bias=nh_ap, scale=-1.0)

                df = workp.tile([128, F], f32, tag="df")
                nc.gpsimd.tensor_tensor(df[:], pu[:], pl[:], ALU.subtract)

                likt = iop.tile([128, F], f32, tag="likt")
                for h in range(2):
                    pout = poutp.tile([128, HF], f32, tag="pout")
                    for k in range(HB):
                        nc.tensor.transpose(
                            pout[:, k * 128:(k + 1) * 128],
                            df[:, h * HF + k * 128: h * HF + (k + 1) * 128],
                            idn[:],
                        )
                    # clip fused with PSUM->SBUF evacuation
                    nc.vector.tensor_scalar(likt[:, h * HF:(h + 1) * HF], pout[:],
                                            1e-9, None, ALU.max)
                nc.sync.dma_start(ls, likt[:])

            c0 = 0
            for _ in range(N_FULL_TILES):
                do_tile(c0, G_FULL)
                c0 += CHUNKS_PER_TILE
            do_tile(c0, G_PART)
            c0 += TILE_F2

            # tail: TAIL_CHUNKS x 128 with partial partitions
            T = TAIL_CHUNKS
            xt = iop.tile([T, 128], f32, tag="xt")
            nc.sync.dma_start(xt[:], x_d[c0:c0 + T, :])
            nt = iop.tile([T, 128], f32, tag="nt")
            nc.sync.dma_start(nt[:], n_d[c0:c0 + T, :])
            ot = iop.tile([T, 128], f32, tag="ot")
            nc.gpsimd.tensor_tensor(ot[:], xt[:], nt[:], ALU.add)
            nc.sync.dma_start(o_d[c0:c0 + T, :], ot[:])

            pin = pinp.tile([128, T], f32, tag="pin")
            nc.tensor.transpose(pin[:], ot[:], idn[:T, :T])
            at = workp.tile([128, T], f32, tag="at")
            nc.scalar.activation(at[:], pin[:], AF.Abs, bias=d_ap, scale=a_ap)
            pu = workp.tile([128, T], f32, tag="pu")
            nc.scalar.activation(pu[:], at[:], AF.Sigmoid, bias=h_ap, scale=-1.0)
            pl = workp.tile([128, T], f32, tag="pl")
            nc.scalar.activation(pl[:], at[:], AF.Sigmoid, bias=nh_ap, scale=-1.0)
            df = workp.tile([128, T], f32, tag="df")
            nc.gpsimd.tensor_tensor(df[:], pu[:], pl[:], ALU.subtract)
            pout = poutp.tile([T, 128], f32, tag="pout")
            nc.tensor.transpose(pout[:], df[:], idn[:, :])
            likt = iop.tile([T, 128], f32, tag="likt")
            nc.vector.tensor_scalar(likt[:], pout[:], 1e-9, None, ALU.max)
            nc.sync.dma_start(lik_d[c0:c0 + T, :], likt[:])

    nc.compile()
    return nc


def _get_nc():
    if "nc" not in _CACHE:
        _CACHE["nc"] = _build_bass()
    return _CACHE["nc"]


def _reference_numpy(inputs):
    """Faithful float32 numpy fallback for the general (f != 0) case."""
    x = np.asarray(inputs["inputs"], dtype=np.float32)
    nz = np.asarray(inputs["noise"], dtype=np.float32)
    o = x + nz
    xt = o.T[:, None, :]  # (C, 1, N)

    def softplus32(v):
        v = v.astype(np.float32)
        return (np.log1p(np.exp(-np.abs(v))) + np.maximum(v, 0)).astype(np.float32)

    def logits_cum(z):
        logits = z.astype(np.float32)
        for i in range(4):
            W = softplus32(np.asarray(inputs[f"m{i}"]))
            b = np.asarray(inputs[f"b{i}"], dtype=np.float32)
            f = np.asarray(inputs[f"f{i}"], dtype=np.float32)
            logits = np.einsum("cij,cjn->cin", W, logits).astype(np.float32) + b
            logits = logits + np.tanh(f) * np.tanh(logits)
        return logits.astype(np.float32)

    lower = logits_cum(xt - np.float32(0.5))
    upper = logits_cum(xt + np.float32(0.5))
    sign = -np.sign(lower + upper)
    def sig(v):
        return (1.0 / (1.0 + np.exp(-v.astype(np.float64)))).astype(np.float32)
    lik = np.abs(sig(sign * upper) - sig(sign * lower))
    lik = lik.reshape(C, -1).T
    lik = np.maximum(lik, np.float32(1e-9))
    return o, lik


def kernel(**inputs):
    x = np.ascontiguousarray(np.asarray(inputs["inputs"], dtype=np.float32))
    nz = np.ascontiguousarray(np.asarray(inputs["noise"], dtype=np.float32))

    f_zero = all(np.all(np.asarray(inputs[f"f{i}"]) == 0) for i in range(4))
    if x.shape != (N_TOTAL, C) or not f_zero:
        return _reference_numpy(inputs)

    a64, d64 = _collapse_affine(inputs)
    a32 = a64.astype(np.float32)
    d32 = d64.astype(np.float32)
    h32 = (0.5 * a64).astype(np.float32)

    prm = np.zeros((128, 4), dtype=np.float32)
    idx = np.arange(128) % C
    prm[:, 0] = a32[idx]
    prm[:, 1] = d32[idx]
    prm[:, 2] = h32[idx]
    prm[:, 3] = -h32[idx]
    idn = np.eye(128, dtype=np.float32)

    nc = _get_nc()
    from concourse.bass_utils import run_bass_kernel_spmd

    xs = x.reshape(N_CORES, CHUNKS, 128)
    ns = nz.reshape(N_CORES, CHUNKS, 128)
    in_maps = [
        {"x": xs[i], "n": ns[i], "prm": prm, "idn": idn}
        for i in range(N_CORES)
    ]
    res = run_bass_kernel_spmd(nc, in_maps, core_ids=list(range(N_CORES)))
    _CACHE["last_results"] = res

    o = np.empty((N_TOTAL, C), dtype=np.float32)
    lik = np.empty((N_TOTAL, C), dtype=np.float32)
    for i, r in enumerate(res.results):
        o[i * ROWS_PER_CORE:(i + 1) * ROWS_PER_CORE] = \
            r["o"].reshape(ROWS_PER_CORE, C)
        lik[i * ROWS_PER_CORE:(i + 1) * ROWS_PER_CORE] = \
            r["lik"].reshape(ROWS_PER_CORE, C)
    return o, lik


# revision 23
# speedup vs baseline: 29.3793x; 29.3793x over previous
"""EntropyBottleneck forward kernel for Trainium2 (8 NeuronCores, data-parallel).

Math: with the per-channel gate params f == 0 (always true for this problem's
inputs), each _logits_cumulative layer is affine, so the whole 4-layer chain
collapses to t = a_c * x + d_c per channel c. The likelihood then is

    lik = | sigmoid(s*(t+h)) - sigmoid(s*(t-h)) |,  s = -sign(2t), h = a_c/2 > 0
        =   sigmoid(-|t| + h) - sigmoid(-|t| - h)   (>= 0, then clipped at 1e-9)

Device work per element: o = x + n (gpsimd), |a*o + d| (ACT Abs w/ per-partition
scale+bias), two sigmoids (ACT w/ per-partition bias), subtract (gpsimd),
clip (DVE, fused with PSUM->SBUF evac). Channels are moved onto the partition
axis with TensorE 128x128 transposes (and back), so all per-channel params are
plain per-partition [128,1] scale/bias vectors and DMAs stay fully contiguous.

Sharding: data-parallel over points N across the 8 cores; tiny params
replicated; no cross-core communication.
"""

import numpy as np

N_TOTAL = 500000
C = 64
N_CORES = 8
ROWS_PER_CORE = N_TOTAL // N_CORES          # 62500
ELEMS = ROWS_PER_CORE * C                   # 4,000,000 per core
CHUNKS = ELEMS // 128                       # 31,250 rows of the [CHUNKS,128] view
G_FULL = 16                                 # 128-col blocks per full tile
TILE_F = G_FULL * 128                       # 2048
CHUNKS_PER_TILE = TILE_F                    # a [128, 2048] tile covers 2048 chunks
N_FULL_TILES = CHUNKS // CHUNKS_PER_TILE    # 15
G_PART = 4
TILE_F2 = G_PART * 128                      # 512; covers 512 chunks
TAIL_CHUNKS = CHUNKS - N_FULL_TILES * CHUNKS_PER_TILE - TILE_F2  # 18

_CACHE: dict = {}


def _softplus64(x):
    return np.log1p(np.exp(-np.abs(x))) + np.maximum(x, 0.0)


def _collapse_affine(inputs):
    """Fold the 4 affine layers into per-channel (a, d) in float64."""
    alpha = None
    beta = None
    for i in range(4):
        W = _softplus64(np.asarray(inputs[f"m{i}"], dtype=np.float64))  # (C, fo, fi)
        bb = np.asarray(inputs[f"b{i}"], dtype=np.float64)[:, :, 0]     # (C, fo)
        if i == 0:
            alpha = W[:, :, 0]
            beta = bb
        else:
            alpha = np.einsum("cij,cj->ci", W, alpha)
            beta = np.einsum("cij,cj->ci", W, beta) + bb
    return alpha[:, 0], beta[:, 0]  # (C,), (C,)


def _build_bass(reps=1, dma_only=False, stage=None, g_full=G_FULL,
                io_bufs=3, work_bufs=2, psum_bufs=2, split_queues=False,
                fine=False, ring_mode="ls"):
    # stage: ablation ladder for perf bisection (None = full kernel):
    #   1 = loads + o-add + stores (lik store carries ot)
    #   2 = + transposes-in + ACT abs (lik store carries at)
    #   3 = + sigmoids + df        (lik store carries df)
    #   None/4 = full kernel
    if dma_only:
        stage = 0
    if stage is None:
        stage = 4
    import concourse.bacc as bacc
    import concourse.mybir as mybir
    from concourse.mybir import ActivationFunctionType as AF
    from concourse.mybir import AluOpType as ALU
    from concourse.tile import TileContext

    f32 = mybir.dt.float32
    nc = bacc.Bacc("TRN2", target_bir_lowering=False, debug=False,
                   enable_asserts=False, num_devices=N_CORES)

    # HWDGE ring assignment: "ls" = loads on SP, stores on ACT ring;
    # "xo_nl" = x-load + o-store on SP, n-load + lik-store on ACT ring;
    # "alt" = per-tile parity alternation; "sw1"/"sw2" = n-load on SWDGE.
    if split_queues and ring_mode == "xo_nl":
        engs = lambda i: (nc.sync, nc.scalar, nc.sync, nc.scalar)
    elif split_queues and ring_mode == "alt":
        engs = lambda i: ((nc.sync, nc.sync, nc.scalar, nc.scalar) if i % 2 == 0
                          else (nc.scalar, nc.scalar, nc.sync, nc.sync))
    elif split_queues and ring_mode == "sw1":
        engs = lambda i: (nc.sync, nc.gpsimd, nc.scalar, nc.scalar)
    elif split_queues and ring_mode == "sw2":
        engs = lambda i: (nc.sync, nc.gpsimd, nc.scalar, nc.sync)
    elif split_queues and ring_mode == "sw4":
        engs = lambda i: (nc.sync, nc.gpsimd, nc.sync, nc.scalar)
    elif split_queues and ring_mode == "sw5":
        engs = lambda i: (nc.gpsimd, nc.sync, nc.scalar, nc.sync)
    elif split_queues and ring_mode == "sw6":
        engs = lambda i: (nc.gpsimd, nc.gpsimd, nc.scalar, nc.sync)
    elif split_queues and ring_mode == "sw7":
        engs = lambda i: (nc.sync, nc.gpsimd, nc.scalar, nc.gpsimd)
    elif split_queues and ring_mode == "sw8":
        engs = lambda i: ((nc.sync, nc.gpsimd, nc.scalar, nc.sync) if i % 2 == 0
                          else (nc.sync, nc.gpsimd, nc.sync, nc.scalar))
    elif split_queues:
        engs = lambda i: (nc.sync, nc.sync, nc.scalar, nc.scalar)
    else:
        engs = lambda i: (nc.sync, nc.sync, nc.sync, nc.sync)
    _tile_counter = [0]
    x_d = nc.dram_tensor("x", [CHUNKS, 128], f32, kind="ExternalInput")
    n_d = nc.dram_tensor("n", [CHUNKS, 128], f32, kind="ExternalInput")
    prm_d = nc.dram_tensor("prm", [128, 4], f32, kind="ExternalInput")
    idn_d = nc.dram_tensor("idn", [128, 128], f32, kind="ExternalInput")
    o_d = nc.dram_tensor("o", [CHUNKS, 128], f32, kind="ExternalOutput")
    lik_d = nc.dram_tensor("lik", [CHUNKS, 128], f32, kind="ExternalOutput")

    with TileContext(nc) as tc:
        with (
            tc.tile_pool(name="const", bufs=1) as constp,
            tc.tile_pool(name="io", bufs=io_bufs) as iop,
            tc.tile_pool(name="work", bufs=work_bufs) as workp,
            tc.tile_pool(name="pin", bufs=psum_bufs, space="PSUM") as pinp,
            tc.tile_pool(name="pout", bufs=psum_bufs, space="PSUM") as poutp,
        ):
            prm = constp.tile([128, 4], f32)
            nc.sync.dma_start(prm[:], prm_d[:, :])
            idn = constp.tile([128, 128], f32)
            nc.sync.dma_start(idn[:], idn_d[:, :])
            a_ap = prm[:, 0:1]
            d_ap = prm[:, 1:2]
            h_ap = prm[:, 2:3]
            nh_ap = prm[:, 3:4]

            def do_tile(c0, g):
                """Process chunks [c0, c0 + g*128) as a [128, g*128] tile."""
                ld_x, ld_n, st_o, st_l = engs(_tile_counter[0])
                _tile_counter[0] += 1
                F = g * 128
                nch = F  # chunks covered
                xs = x_d[c0:c0 + nch, :].rearrange("(q g) j -> q (g j)", q=128)
                ns = n_d[c0:c0 + nch, :].rearrange("(q g) j -> q (g j)", q=128)
                os = o_d[c0:c0 + nch, :].rearrange("(q g) j -> q (g j)", q=128)
                ls = lik_d[c0:c0 + nch, :].rearrange("(q g) j -> q (g j)", q=128)

                xt = iop.tile([128, F], f32, tag="xt")
                ld_x.dma_start(xt[:], xs)
                nt = iop.tile([128, F], f32, tag="nt")
                ld_n.dma_start(nt[:], ns)

                if stage == 0:
                    st_o.dma_start(os, xt[:])
                    st_l.dma_start(ls, nt[:])
                    return

                # o = x + n, split across DVE / GPSIMD to balance engine load
                ot = iop.tile([128, F], f32, tag="ot")
                MF = F // 2
                nc.vector.tensor_tensor(ot[:, 0:MF], xt[:, 0:MF], nt[:, 0:MF],
                                        ALU.add)
                nc.gpsimd.tensor_tensor(ot[:, MF:F], xt[:, MF:F], nt[:, MF:F],
                                        ALU.add)
                if fine:
                    st_o.dma_start(os[:, 0:MF], ot[:, 0:MF])
                    st_o.dma_start(os[:, MF:F], ot[:, MF:F])
                else:
                    st_o.dma_start(os, ot[:])
                if stage == 1:
                    st_l.dma_start(ls, ot[:])
                    return

                at = workp.tile([128, F], f32, tag="at")
                HB = g // 2  # 128-blocks per PSUM half-tile
                HF = HB * 128
                for h in range(2):
                    pin = pinp.tile([128, HF], f32, tag="pin")
                    for k in range(HB):
                        nc.tensor.transpose(
                            pin[:, k * 128:(k + 1) * 128],
                            ot[:, h * HF + k * 128: h * HF + (k + 1) * 128],
                            idn[:],
                        )
                    # at = |a * oT + d|, per-partition scale/bias
                    nc.scalar.activation(at[:, h * HF:(h + 1) * HF], pin[:],
                                         AF.Abs, bias=d_ap, scale=a_ap)
                if stage == 2:
                    st_l.dma_start(ls, at[:])
                    return

                pu = workp.tile([128, F], f32, tag="pu")
                nc.scalar.activation(pu[:], at[:], AF.Sigmoid, bias=h_ap, scale=-1.0)
                pl = workp.tile([128, F], f32, tag="pl")
                nc.scalar.activation(pl[:], at[:], AF.Sigmoid, bias=nh_ap, scale=-1.0)

                df = workp.tile([128, F], f32, tag="df")
                if fine:
                    nc.gpsimd.tensor_tensor(df[:, 0:MF], pu[:, 0:MF],
                                            pl[:, 0:MF], ALU.subtract)
                    nc.gpsimd.tensor_tensor(df[:, MF:F], pu[:, MF:F],
                                            pl[:, MF:F], ALU.subtract)
                else:
                    nc.gpsimd.tensor_tensor(df[:], pu[:], pl[:], ALU.subtract)
                if stage == 3:
                    st_l.dma_start(ls, df[:])
                    return

                likt = iop.tile([128, F], f32, tag="likt")
                for h in range(2):
                    pout = poutp.tile([128, HF], f32, tag="pout")
                    for k in range(HB):
                        nc.tensor.transpose(
                            pout[:, k * 128:(k + 1) * 128],
                            df[:, h * HF + k * 128: h * HF + (k + 1) * 128],
                            idn[:],
                        )
                    # clip fused with PSUM->SBUF evacuation
                    nc.vector.tensor_scalar(likt[:, h * HF:(h + 1) * HF], pout[:],
                                            1e-9, None, ALU.max)
                st_l.dma_start(ls, likt[:])

            def do_tail(c0):
                ld_x, ld_n, st_o, st_l = engs(_tile_counter[0])
                _tile_counter[0] += 1
                T = TAIL_CHUNKS
                if stage < 4:
                    xt = iop.tile([T, 128], f32, tag="xt")
                    nc.sync.dma_start(xt[:], x_d[c0:c0 + T, :])
                    nt = iop.tile([T, 128], f32, tag="nt")
                    nc.sync.dma_start(nt[:], n_d[c0:c0 + T, :])
                    st_o.dma_start(o_d[c0:c0 + T, :], xt[:])
                    st_l.dma_start(lik_d[c0:c0 + T, :], nt[:])
                    return
                xt = iop.tile([T, 128], f32, tag="xt")
                nc.sync.dma_start(xt[:], x_d[c0:c0 + T, :])
                nt = iop.tile([T, 128], f32, tag="nt")
                nc.sync.dma_start(nt[:], n_d[c0:c0 + T, :])
                ot = iop.tile([T, 128], f32, tag="ot")
                nc.gpsimd.tensor_tensor(ot[:], xt[:], nt[:], ALU.add)
                st_o.dma_start(o_d[c0:c0 + T, :], ot[:])

                pin = pinp.tile([128, T], f32, tag="pin")
                nc.tensor.transpose(pin[:], ot[:], idn[:T, :T])
                at = workp.tile([128, T], f32, tag="at")
                nc.scalar.activation(at[:], pin[:], AF.Abs, bias=d_ap, scale=a_ap)
                pu = workp.tile([128, T], f32, tag="pu")
                nc.scalar.activation(pu[:], at[:], AF.Sigmoid, bias=h_ap, scale=-1.0)
                pl = workp.tile([128, T], f32, tag="pl")
                nc.scalar.activation(pl[:], at[:], AF.Sigmoid, bias=nh_ap, scale=-1.0)
                df = workp.tile([128, T], f32, tag="df")
                nc.gpsimd.tensor_tensor(df[:], pu[:], pl[:], ALU.subtract)
                pout = poutp.tile([T, 128], f32, tag="pout")
                nc.tensor.transpose(pout[:], df[:], idn[:, :])
                likt = iop.tile([T, 128], f32, tag="likt")
                nc.vector.tensor_scalar(likt[:], pout[:], 1e-9, None, ALU.max)
                st_l.dma_start(lik_d[c0:c0 + T, :], likt[:])

            main_chunks = CHUNKS - TAIL_CHUNKS          # 31232, multiple of 512
            n_full = main_chunks // (g_full * 128)
            leftover = main_chunks - n_full * g_full * 128
            assert leftover % (G_PART * 128) == 0
            for _ in range(reps):
                c0 = 0
                for _ in range(n_full):
                    do_tile(c0, g_full)
                    c0 += g_full * 128
                while c0 < main_chunks:
                    do_tile(c0, G_PART)
                    c0 += G_PART * 128
                do_tail(c0)

    nc.compile()
    return nc


def _get_nc():
    if "nc" not in _CACHE:
        _CACHE["nc"] = _build_bass(split_queues=True, ring_mode="sw7")
    return _CACHE["nc"]


def _reference_numpy(inputs):
    """Faithful float32 numpy fallback for the general (f != 0) case."""
    x = np.asarray(inputs["inputs"], dtype=np.float32)
    nz = np.asarray(inputs["noise"], dtype=np.float32)
    o = x + nz
    xt = o.T[:, None, :]  # (C, 1, N)

    def softplus32(v):
        v = v.astype(np.float32)
        return (np.log1p(np.exp(-np.abs(v))) + np.maximum(v, 0)).astype(np.float32)

    def logits_cum(z):
        logits = z.astype(np.float32)
        for i in range(4):
            W = softplus32(np.asarray(inputs[f"m{i}"]))
            b = np.asarray(inputs[f"b{i}"], dtype=np.float32)
            f = np.asarray(inputs[f"f{i}"], dtype=np.float32)
            logits = np.einsum("cij,cjn->cin", W, logits).astype(np.float32) + b
            logits = logits + np.tanh(f) * np.tanh(logits)
        return logits.astype(np.float32)

    lower = logits_cum(xt - np.float32(0.5))
    upper = logits_cum(xt + np.float32(0.5))
    sign = -np.sign(lower + upper)
    def sig(v):
        return (1.0 / (1.0 + np.exp(-v.astype(np.float64)))).astype(np.float32)
    lik = np.abs(sig(sign * upper) - sig(sign * lower))
    lik = lik.reshape(C, -1).T
    lik = np.maximum(lik, np.float32(1e-9))
    return o, lik


def kernel(**inputs):
    x = np.ascontiguousarray(np.asarray(inputs["inputs"], dtype=np.float32))
    nz = np.ascontiguousarray(np.asarray(inputs["noise"], dtype=np.float32))

    f_zero = all(np.all(np.asarray(inputs[f"f{i}"]) == 0) for i in range(4))
    if x.shape != (N_TOTAL, C) or not f_zero:
        return _reference_numpy(inputs)

    a64, d64 = _collapse_affine(inputs)
    a32 = a64.astype(np.float32)
    d32 = d64.astype(np.float32)
    h32 = (0.5 * a64).astype(np.float32)

    prm = np.zeros((128, 4), dtype=np.float32)
    idx = np.arange(128) % C
    prm[:, 0] = a32[idx]
    prm[:, 1] = d32[idx]
    prm[:, 2] = h32[idx]
    prm[:, 3] = -h32[idx]
    idn = np.eye(128, dtype=np.float32)

    nc = _get_nc()
    from concourse.bass_utils import run_bass_kernel_spmd

    xs = x.reshape(N_CORES, CHUNKS, 128)
    ns = nz.reshape(N_CORES, CHUNKS, 128)
    in_maps = [
        {"x": xs[i], "n": ns[i], "prm": prm, "idn": idn}
        for i in range(N_CORES)
    ]
    res = run_bass_kernel_spmd(nc, in_maps, core_ids=list(range(N_CORES)))
    _CACHE["last_results"] = res

    o = np.empty((N_TOTAL, C), dtype=np.float32)
    lik = np.empty((N_TOTAL, C), dtype=np.float32)
    for i, r in enumerate(res.results):
        o[i * ROWS_PER_CORE:(i + 1) * ROWS_PER_CORE] = \
            r["o"].reshape(ROWS_PER_CORE, C)
        lik[i * ROWS_PER_CORE:(i + 1) * ROWS_PER_CORE] = \
            r["lik"].reshape(ROWS_PER_CORE, C)
    return o, lik


# revision 25
# speedup vs baseline: 31.5471x; 1.0738x over previous
"""EntropyBottleneck forward kernel for Trainium2 (8 NeuronCores, data-parallel).

Math: with the per-channel gate params f == 0 (always true for this problem's
inputs), each _logits_cumulative layer is affine, so the whole 4-layer chain
collapses to t = a_c * x + d_c per channel c. The likelihood then is

    lik = | sigmoid(s*(t+h)) - sigmoid(s*(t-h)) |,  s = -sign(2t), h = a_c/2 > 0
        =   sigmoid(-|t| + h) - sigmoid(-|t| - h)   (>= 0, then clipped at 1e-9)

Device work per element: o = x + n (gpsimd), |a*o + d| (ACT Abs w/ per-partition
scale+bias), two sigmoids (ACT w/ per-partition bias), subtract (gpsimd),
clip (DVE, fused with PSUM->SBUF evac). Channels are moved onto the partition
axis with TensorE 128x128 transposes (and back), so all per-channel params are
plain per-partition [128,1] scale/bias vectors and DMAs stay fully contiguous.

Sharding: data-parallel over points N across the 8 cores; tiny params
replicated; no cross-core communication.
"""

import numpy as np

N_TOTAL = 500000
C = 64
N_CORES = 8
ROWS_PER_CORE = N_TOTAL // N_CORES          # 62500
ELEMS = ROWS_PER_CORE * C                   # 4,000,000 per core
CHUNKS = ELEMS // 128                       # 31,250 rows of the [CHUNKS,128] view
G_FULL = 16                                 # 128-col blocks per full tile
TILE_F = G_FULL * 128                       # 2048
CHUNKS_PER_TILE = TILE_F                    # a [128, 2048] tile covers 2048 chunks
N_FULL_TILES = CHUNKS // CHUNKS_PER_TILE    # 15
G_PART = 4
TILE_F2 = G_PART * 128                      # 512; covers 512 chunks
TAIL_CHUNKS = CHUNKS - N_FULL_TILES * CHUNKS_PER_TILE - TILE_F2  # 18

_CACHE: dict = {}


def _softplus64(x):
    return np.log1p(np.exp(-np.abs(x))) + np.maximum(x, 0.0)


def _collapse_affine(inputs):
    """Fold the 4 affine layers into per-channel (a, d) in float64."""
    alpha = None
    beta = None
    for i in range(4):
        W = _softplus64(np.asarray(inputs[f"m{i}"], dtype=np.float64))  # (C, fo, fi)
        bb = np.asarray(inputs[f"b{i}"], dtype=np.float64)[:, :, 0]     # (C, fo)
        if i == 0:
            alpha = W[:, :, 0]
            beta = bb
        else:
            alpha = np.einsum("cij,cj->ci", W, alpha)
            beta = np.einsum("cij,cj->ci", W, beta) + bb
    return alpha[:, 0], beta[:, 0]  # (C,), (C,)


def _build_bass(reps=1, dma_only=False, stage=None, g_full=G_FULL,
                io_bufs=3, work_bufs=2, psum_bufs=2, split_queues=False,
                fine=False, ring_mode="ls", add_mode="split"):
    # stage: ablation ladder for perf bisection (None = full kernel):
    #   1 = loads + o-add + stores (lik store carries ot)
    #   2 = + transposes-in + ACT abs (lik store carries at)
    #   3 = + sigmoids + df        (lik store carries df)
    #   None/4 = full kernel
    if dma_only:
        stage = 0
    if stage is None:
        stage = 4
    import concourse.bacc as bacc
    import concourse.mybir as mybir
    from concourse.mybir import ActivationFunctionType as AF
    from concourse.mybir import AluOpType as ALU
    from concourse.tile import TileContext

    f32 = mybir.dt.float32
    nc = bacc.Bacc("TRN2", target_bir_lowering=False, debug=False,
                   enable_asserts=False, num_devices=N_CORES)

    # HWDGE ring assignment: "ls" = loads on SP, stores on ACT ring;
    # "xo_nl" = x-load + o-store on SP, n-load + lik-store on ACT ring;
    # "alt" = per-tile parity alternation; "sw1"/"sw2" = n-load on SWDGE.
    if split_queues and ring_mode == "xo_nl":
        engs = lambda i: (nc.sync, nc.scalar, nc.sync, nc.scalar)
    elif split_queues and ring_mode == "alt":
        engs = lambda i: ((nc.sync, nc.sync, nc.scalar, nc.scalar) if i % 2 == 0
                          else (nc.scalar, nc.scalar, nc.sync, nc.sync))
    elif split_queues and ring_mode == "sw1":
        engs = lambda i: (nc.sync, nc.gpsimd, nc.scalar, nc.scalar)
    elif split_queues and ring_mode == "sw2":
        engs = lambda i: (nc.sync, nc.gpsimd, nc.scalar, nc.sync)
    elif split_queues and ring_mode == "sw4":
        engs = lambda i: (nc.sync, nc.gpsimd, nc.sync, nc.scalar)
    elif split_queues and ring_mode == "sw5":
        engs = lambda i: (nc.gpsimd, nc.sync, nc.scalar, nc.sync)
    elif split_queues and ring_mode == "sw6":
        engs = lambda i: (nc.gpsimd, nc.gpsimd, nc.scalar, nc.sync)
    elif split_queues and ring_mode == "sw7":
        engs = lambda i: (nc.sync, nc.gpsimd, nc.scalar, nc.gpsimd)
    elif split_queues and ring_mode == "sw8":
        engs = lambda i: ((nc.sync, nc.gpsimd, nc.scalar, nc.sync) if i % 2 == 0
                          else (nc.sync, nc.gpsimd, nc.sync, nc.scalar))
    elif split_queues:
        engs = lambda i: (nc.sync, nc.sync, nc.scalar, nc.scalar)
    else:
        engs = lambda i: (nc.sync, nc.sync, nc.sync, nc.sync)
    _tile_counter = [0]
    x_d = nc.dram_tensor("x", [CHUNKS, 128], f32, kind="ExternalInput")
    n_d = nc.dram_tensor("n", [CHUNKS, 128], f32, kind="ExternalInput")
    prm_d = nc.dram_tensor("prm", [128, 4], f32, kind="ExternalInput")
    idn_d = nc.dram_tensor("idn", [128, 128], f32, kind="ExternalInput")
    o_d = nc.dram_tensor("o", [CHUNKS, 128], f32, kind="ExternalOutput")
    lik_d = nc.dram_tensor("lik", [CHUNKS, 128], f32, kind="ExternalOutput")

    with TileContext(nc) as tc:
        with (
            tc.tile_pool(name="const", bufs=1) as constp,
            tc.tile_pool(name="io", bufs=io_bufs) as iop,
            tc.tile_pool(name="work", bufs=work_bufs) as workp,
            tc.tile_pool(name="pin", bufs=psum_bufs, space="PSUM") as pinp,
            tc.tile_pool(name="pout", bufs=psum_bufs, space="PSUM") as poutp,
        ):
            prm = constp.tile([128, 4], f32)
            nc.sync.dma_start(prm[:], prm_d[:, :])
            idn = constp.tile([128, 128], f32)
            nc.sync.dma_start(idn[:], idn_d[:, :])
            a_ap = prm[:, 0:1]
            d_ap = prm[:, 1:2]
            h_ap = prm[:, 2:3]
            nh_ap = prm[:, 3:4]

            def do_tile(c0, g):
                """Process chunks [c0, c0 + g*128) as a [128, g*128] tile."""
                ld_x, ld_n, st_o, st_l = engs(_tile_counter[0])
                _tile_counter[0] += 1
                F = g * 128
                nch = F  # chunks covered
                xs = x_d[c0:c0 + nch, :].rearrange("(q g) j -> q (g j)", q=128)
                ns = n_d[c0:c0 + nch, :].rearrange("(q g) j -> q (g j)", q=128)
                os = o_d[c0:c0 + nch, :].rearrange("(q g) j -> q (g j)", q=128)
                ls = lik_d[c0:c0 + nch, :].rearrange("(q g) j -> q (g j)", q=128)

                xt = iop.tile([128, F], f32, tag="xt")
                ld_x.dma_start(xt[:], xs)
                nt = iop.tile([128, F], f32, tag="nt")
                ld_n.dma_start(nt[:], ns)

                if stage == 0:
                    st_o.dma_start(os, xt[:])
                    st_l.dma_start(ls, nt[:])
                    return

                # o = x + n, split across DVE / GPSIMD to balance engine load
                ot = iop.tile([128, F], f32, tag="ot")
                MF = F // 2
                if add_mode == "dve":
                    nc.vector.tensor_tensor(ot[:], xt[:], nt[:], ALU.add)
                else:
                    nc.vector.tensor_tensor(ot[:, 0:MF], xt[:, 0:MF],
                                            nt[:, 0:MF], ALU.add)
                    nc.gpsimd.tensor_tensor(ot[:, MF:F], xt[:, MF:F],
                                            nt[:, MF:F], ALU.add)
                if fine:
                    st_o.dma_start(os[:, 0:MF], ot[:, 0:MF])
                    st_o.dma_start(os[:, MF:F], ot[:, MF:F])
                else:
                    st_o.dma_start(os, ot[:])
                if stage == 1:
                    st_l.dma_start(ls, ot[:])
                    return

                at = workp.tile([128, F], f32, tag="at")
                HB = g // 2  # 128-blocks per PSUM half-tile
                HF = HB * 128
                for h in range(2):
                    pin = pinp.tile([128, HF], f32, tag="pin")
                    for k in range(HB):
                        nc.tensor.transpose(
                            pin[:, k * 128:(k + 1) * 128],
                            ot[:, h * HF + k * 128: h * HF + (k + 1) * 128],
                            idn[:],
                        )
                    # at = |a * oT + d|, per-partition scale/bias
                    nc.scalar.activation(at[:, h * HF:(h + 1) * HF], pin[:],
                                         AF.Abs, bias=d_ap, scale=a_ap)
                if stage == 2:
                    st_l.dma_start(ls, at[:])
                    return

                pu = workp.tile([128, F], f32, tag="pu")
                nc.scalar.activation(pu[:], at[:], AF.Sigmoid, bias=h_ap, scale=-1.0)
                pl = workp.tile([128, F], f32, tag="pl")
                nc.scalar.activation(pl[:], at[:], AF.Sigmoid, bias=nh_ap, scale=-1.0)

                df = workp.tile([128, F], f32, tag="df")
                if fine:
                    nc.gpsimd.tensor_tensor(df[:, 0:MF], pu[:, 0:MF],
                                            pl[:, 0:MF], ALU.subtract)
                    nc.gpsimd.tensor_tensor(df[:, MF:F], pu[:, MF:F],
                                            pl[:, MF:F], ALU.subtract)
                else:
                    nc.gpsimd.tensor_tensor(df[:], pu[:], pl[:], ALU.subtract)
                if stage == 3:
                    st_l.dma_start(ls, df[:])
                    return

                likt = iop.tile([128, F], f32, tag="likt")
                for h in range(2):
                    pout = poutp.tile([128, HF], f32, tag="pout")
                    for k in range(HB):
                        nc.tensor.transpose(
                            pout[:, k * 128:(k + 1) * 128],
                            df[:, h * HF + k * 128: h * HF + (k + 1) * 128],
                            idn[:],
                        )
                    # clip fused with PSUM->SBUF evacuation
                    nc.vector.tensor_scalar(likt[:, h * HF:(h + 1) * HF], pout[:],
                                            1e-9, None, ALU.max)
                st_l.dma_start(ls, likt[:])

            def do_tail(c0):
                ld_x, ld_n, st_o, st_l = engs(_tile_counter[0])
                _tile_counter[0] += 1
                T = TAIL_CHUNKS
                if stage < 4:
                    xt = iop.tile([T, 128], f32, tag="xt")
                    nc.sync.dma_start(xt[:], x_d[c0:c0 + T, :])
                    nt = iop.tile([T, 128], f32, tag="nt")
                    nc.sync.dma_start(nt[:], n_d[c0:c0 + T, :])
                    st_o.dma_start(o_d[c0:c0 + T, :], xt[:])
                    st_l.dma_start(lik_d[c0:c0 + T, :], nt[:])
                    return
                xt = iop.tile([T, 128], f32, tag="xt")
                nc.sync.dma_start(xt[:], x_d[c0:c0 + T, :])
                nt = iop.tile([T, 128], f32, tag="nt")
                nc.sync.dma_start(nt[:], n_d[c0:c0 + T, :])
                ot = iop.tile([T, 128], f32, tag="ot")
                nc.gpsimd.tensor_tensor(ot[:], xt[:], nt[:], ALU.add)
                st_o.dma_start(o_d[c0:c0 + T, :], ot[:])

                pin = pinp.tile([128, T], f32, tag="pin")
                nc.tensor.transpose(pin[:], ot[:], idn[:T, :T])
                at = workp.tile([128, T], f32, tag="at")
                nc.scalar.activation(at[:], pin[:], AF.Abs, bias=d_ap, scale=a_ap)
                pu = workp.tile([128, T], f32, tag="pu")
                nc.scalar.activation(pu[:], at[:], AF.Sigmoid, bias=h_ap, scale=-1.0)
                pl = workp.tile([128, T], f32, tag="pl")
                nc.scalar.activation(pl[:], at[:], AF.Sigmoid, bias=nh_ap, scale=-1.0)
                df = workp.tile([128, T], f32, tag="df")
                nc.gpsimd.tensor_tensor(df[:], pu[:], pl[:], ALU.subtract)
                pout = poutp.tile([T, 128], f32, tag="pout")
                nc.tensor.transpose(pout[:], df[:], idn[:, :])
                likt = iop.tile([T, 128], f32, tag="likt")
                nc.vector.tensor_scalar(likt[:], pout[:], 1e-9, None, ALU.max)
                st_l.dma_start(lik_d[c0:c0 + T, :], likt[:])

            main_chunks = CHUNKS - TAIL_CHUNKS          # 31232, multiple of 512
            n_full = main_chunks // (g_full * 128)
            leftover = main_chunks - n_full * g_full * 128
            assert leftover % (G_PART * 128) == 0
            for _ in range(reps):
                c0 = 0
                for _ in range(n_full):
                    do_tile(c0, g_full)
                    c0 += g_full * 128
                while c0 < main_chunks:
                    do_tile(c0, G_PART)
                    c0 += G_PART * 128
                do_tail(c0)

    nc.compile()
    return nc


def _get_nc():
    if "nc" not in _CACHE:
        _CACHE["nc"] = _build_bass(split_queues=True, ring_mode="sw7", add_mode="dve")
    return _CACHE["nc"]


def _reference_numpy(inputs):
    """Faithful float32 numpy fallback for the general (f != 0) case."""
    x = np.asarray(inputs["inputs"], dtype=np.float32)
    nz = np.asarray(inputs["noise"], dtype=np.float32)
    o = x + nz
    xt = o.T[:, None, :]  # (C, 1, N)

    def softplus32(v):
        v = v.astype(np.float32)
        return (np.log1p(np.exp(-np.abs(v))) + np.maximum(v, 0)).astype(np.float32)

    def logits_cum(z):
        logits = z.astype(np.float32)
        for i in range(4):
            W = softplus32(np.asarray(inputs[f"m{i}"]))
            b = np.asarray(inputs[f"b{i}"], dtype=np.float32)
            f = np.asarray(inputs[f"f{i}"], dtype=np.float32)
            logits = np.einsum("cij,cjn->cin", W, logits).astype(np.float32) + b
            logits = logits + np.tanh(f) * np.tanh(logits)
        return logits.astype(np.float32)

    lower = logits_cum(xt - np.float32(0.5))
    upper = logits_cum(xt + np.float32(0.5))
    sign = -np.sign(lower + upper)
    def sig(v):
        return (1.0 / (1.0 + np.exp(-v.astype(np.float64)))).astype(np.float32)
    lik = np.abs(sig(sign * upper) - sig(sign * lower))
    lik = lik.reshape(C, -1).T
    lik = np.maximum(lik, np.float32(1e-9))
    return o, lik


def kernel(**inputs):
    x = np.ascontiguousarray(np.asarray(inputs["inputs"], dtype=np.float32))
    nz = np.ascontiguousarray(np.asarray(inputs["noise"], dtype=np.float32))

    f_zero = all(np.all(np.asarray(inputs[f"f{i}"]) == 0) for i in range(4))
    if x.shape != (N_TOTAL, C) or not f_zero:
        return _reference_numpy(inputs)

    a64, d64 = _collapse_affine(inputs)
    a32 = a64.astype(np.float32)
    d32 = d64.astype(np.float32)
    h32 = (0.5 * a64).astype(np.float32)

    prm = np.zeros((128, 4), dtype=np.float32)
    idx = np.arange(128) % C
    prm[:, 0] = a32[idx]
    prm[:, 1] = d32[idx]
    prm[:, 2] = h32[idx]
    prm[:, 3] = -h32[idx]
    idn = np.eye(128, dtype=np.float32)

    nc = _get_nc()
    from concourse.bass_utils import run_bass_kernel_spmd

    xs = x.reshape(N_CORES, CHUNKS, 128)
    ns = nz.reshape(N_CORES, CHUNKS, 128)
    in_maps = [
        {"x": xs[i], "n": ns[i], "prm": prm, "idn": idn}
        for i in range(N_CORES)
    ]
    res = run_bass_kernel_spmd(nc, in_maps, core_ids=list(range(N_CORES)))
    _CACHE["last_results"] = res

    o = np.empty((N_TOTAL, C), dtype=np.float32)
    lik = np.empty((N_TOTAL, C), dtype=np.float32)
    for i, r in enumerate(res.results):
        o[i * ROWS_PER_CORE:(i + 1) * ROWS_PER_CORE] = \
            r["o"].reshape(ROWS_PER_CORE, C)
        lik[i * ROWS_PER_CORE:(i + 1) * ROWS_PER_CORE] = \
            r["lik"].reshape(ROWS_PER_CORE, C)
    return o, lik
